# revision 39
# baseline (speedup 1.0000x reference)
"""
Multi-head attention + residual + LayerNorm Trainium2 kernel (8 NeuronCores).

Problem (hardcoded shapes):
    hidden_states [2, 2048, 1024] f32, mask [2, 2048, 2048] int32,
    Wq/Wk/Wv/Wd [1024, 1024] f32, bd/gamma/beta [1024] f32.
    out = LayerNorm(ctx @ Wd.T + bd + hidden_states) with 16 heads, hd=64.

Sharding: pure data parallel. Core c handles batch b = c//4 and query rows
q in [ (c%4)*512, (c%4)*512+512 ).  Each core computes K/V for the full
sequence of its batch (4x redundant), attention + dense + LN for its own
512 rows.  No collectives.

Key speed tricks vs the earlier revision:
  * Q/K/V and dense projections run in fp8(e4m3) with DoubleRow perf mode
    (2 contraction chunks per matmul, ~1.7x effective PE rate) -- the
    attention path stays bf16 (probs can't survive fp8 range).
  * Attention processes HEAD PAIRS with the even head's score matmuls on
    PE row-tile (0,0) and the odd head's on (64,0) (contract=64), so bass's
    auto tile_position makes consecutive even/odd matmuls run CONCURRENTLY
    on the two halves of the PE array: 2x on the scores stage.
  * Softmax normalizer via an all-ones column appended to V (row 64 of the
    transposed ctx = sum of masked probs); normalization per head pair as
    soon as its ctx finishes (reciprocal + tiny selector matmul broadcast),
    so the dense phase starts without a serialization bubble.  The dense
    contracts chunk pair (6,7) last so it can start before pair 7 is
    normalized.
  * Startup: a ~40-matmul warmup block on a memset tile keeps the PE busy
    from t=0 (HAM clock gate stays at 2.4 GHz) while fp8 inputs stream in,
    fine-grained and priority-ordered across the sync/scalar/gpsimd DMA
    queues (first real matmul needs only 2 small transfers).
  * Mask multiplies alternate DVE (even head) / GpSimd (odd head) to keep
    the vector engine off the critical path; output DMA is split across
    two queues per row block.
"""

import os
import sys
from contextlib import ExitStack

import numpy as np

for _p in ("/opt/trn_rl_repo",):
    if os.path.isdir(_p) and _p not in sys.path:
        sys.path.insert(0, _p)

import ml_dtypes  # noqa: E402

import concourse.bass as bass  # noqa: E402
import concourse.tile as tile  # noqa: E402
from concourse import bacc, mybir  # noqa: E402
from concourse.bass_utils import run_bass_kernel_spmd  # noqa: E402

BF16 = mybir.dt.bfloat16
F32 = mybir.dt.float32
FP8 = mybir.dt.float8e4
DR = mybir.MatmulPerfMode.DoubleRow
NP_BF16 = ml_dtypes.bfloat16
NP_FP8 = ml_dtypes.float8_e4m3

B, S, H, NH = 2, 2048, 1024, 16
HD = H // NH  # 64
P = 128
NCORES = 8
SQ = S // 4  # 512 query rows per core
FC = H // P  # 8 feature chunks
KC = S // P  # 16 kv chunks
SCALE = 1.0 / float(np.sqrt(HD))
EPS = 1e-6
NWARM = 56

# Results of the last device run (for test harness introspection)
last_results = None


def _build_program(affine=True):
    nc = bacc.Bacc(
        "TRN2",
        target_bir_lowering=False,
        debug=False,
        enable_asserts=False,
        num_devices=NCORES,
    )

    # Per-core DRAM inputs (fp8 operands pre-laid-out in SBUF image order)
    d_xT = nc.dram_tensor("xT", [FC, P, S], FP8, kind="ExternalInput").ap()
    d_wq = nc.dram_tensor("wqT", [P, FC, FC, P], FP8, kind="ExternalInput").ap()
    d_wk = nc.dram_tensor("wkT", [P, FC, FC, P], FP8, kind="ExternalInput").ap()
    d_wv = nc.dram_tensor("wvT", [P, 2, FC, 512], FP8, kind="ExternalInput").ap()
    d_wd = nc.dram_tensor("wdT", [P, 2, FC, 512], FP8, kind="ExternalInput").ap()
    d_maskT = nc.dram_tensor("maskT", [KC, P, SQ], BF16, kind="ExternalInput").ap()
    d_xres = nc.dram_tensor("xres", [SQ // P, P, H], F32, kind="ExternalInput").ap()
    d_gamma = nc.dram_tensor("gamma", [H], F32, kind="ExternalInput").ap()
    d_beta = nc.dram_tensor("beta", [H], F32, kind="ExternalInput").ap()
    d_sel = nc.dram_tensor("sel", [2, P], F32, kind="ExternalInput").ap()
    d_out = nc.dram_tensor("out", [SQ // P, P, H], F32, kind="ExternalOutput").ap()
    d_sums = nc.dram_tensor("sums_scratch", [FC, 2, SQ], F32, kind="Internal").ap()

    with tile.TileContext(nc, trace_sim=False) as tc:
        _program(tc, d_xT, d_wq, d_wk, d_wv, d_wd, d_maskT, d_xres, d_gamma,
                 d_beta, d_sel, d_out, d_sums, affine)

    nc.compile()
    return nc


def _bcast_ap(src_1d, parts):
    """AP that replicates a [n] DRAM vector across `parts` partitions."""
    return bass.AP(
        tensor=src_1d.tensor,
        offset=src_1d.offset,
        ap=[[0, parts]] + list(src_1d.ap),
    )


def _program(ctx_or_tc, *args):
    with ExitStack() as ctx:
        _program_inner(ctx, ctx_or_tc, *args)


def _program_inner(ctx, tc, d_xT, d_wq, d_wk, d_wv, d_wd, d_maskT, d_xres,
                   d_gamma, d_beta, d_sel, d_out, d_sums, affine):
    from collections import deque
    nc = tc.nc

    # ---------------- pools ----------------
    persist = ctx.enter_context(tc.tile_pool(name="persist", bufs=1))
    ps_s = ctx.enter_context(tc.tile_pool(name="ps_s", bufs=1, space="PSUM"))
    ps_c = ctx.enter_context(tc.tile_pool(name="ps_c", bufs=1, space="PSUM"))
    ps_mm = ctx.enter_context(tc.tile_pool(name="ps_mm", bufs=2, space="PSUM"))

    # ---------------- persistent tiles ----------------
    kT_hp = [persist.tile([P, S], BF16, name=f"kT{hp}") for hp in range(FC)]
    qT_hp = [persist.tile([P, SQ], BF16, name=f"qT{hp}") for hp in range(FC)]
    v_g = [persist.tile([P, KC, 8, HD + 1], BF16, name=f"v{g}") for g in range(2)]
    ctxT_sb = persist.tile([P, FC, SQ], BF16)  # raw ctx drains
    ctxF8 = persist.tile([P, FC, SQ], FP8)     # normalized, fp8 for dense
    maskT_sb = persist.tile([P, KC, SQ], BF16)
    sel2 = persist.tile([2, P], F32)
    xres_sb = persist.tile([P, SQ // P, H], F32)
    warm_src = persist.tile([P, 512], BF16)

    nc.vector.memset(warm_src, 0.0)
    for g in range(2):
        nc.vector.memset(v_g[g][:, :, :, HD : HD + 1], 1.0)

    # PE warmup: a dependency-free accumulation chain keeps the PE busy (and
    # the HAM clock un-throttled) while the first input DMAs stream in.
    warm_ps = ps_mm.tile([P, 512], F32, name="warm", tag="mm")
    for i in range(NWARM):
        nc.tensor.matmul(warm_ps, lhsT=warm_src[:, 0:P], rhs=warm_src,
                         start=(i == 0), stop=(i == NWARM - 1))

    work = ctx.enter_context(tc.tile_pool(name="work", bufs=3))

    # ---------------- input DMAs, priority-ordered per queue ----------------
    from contextlib import ExitStack as _ES
    proj_ctx = _ES()
    pool_xt = proj_ctx.enter_context(tc.tile_pool(name="proj_xt", bufs=1))
    xT_sb = pool_xt.tile([P, FC, S], FP8)
    pool_w = proj_ctx.enter_context(tc.tile_pool(name="proj_w", bufs=1))
    wq_sb = pool_w.tile([P, FC, FC, P], FP8)
    wk_sb = pool_w.tile([P, FC, FC, P], FP8)
    wv_sb = pool_w.tile([P, 2, FC, 512], FP8)

    # gpsimd: small high-priority weights, then v-weights, then mask chunks
    nc.gpsimd.dma_start(out=wq_sb[:, 0], in_=d_wq[:, 0])
    nc.gpsimd.dma_start(out=wk_sb[:, 0], in_=d_wk[:, 0])
    nc.gpsimd.dma_start(out=wv_sb[:, 0], in_=d_wv[:, 0])
    nc.gpsimd.dma_start(out=wv_sb[:, 1], in_=d_wv[:, 1])
    for mq in range(4):
        nc.gpsimd.dma_start(
            out=maskT_sb[:, 4 * mq : 4 * mq + 4, :],
            in_=d_maskT[4 * mq : 4 * mq + 4].rearrange("c p n -> p c n"))
    nc.gpsimd.dma_start(out=sel2, in_=d_sel)
    # sync: even xT chunks, then remaining q-weights, then residual rows
    # scalar: odd xT chunks, then remaining k-weights
    for c in range(FC):
        eng = nc.sync if c % 2 == 0 else nc.scalar
        eng.dma_start(out=xT_sb[:, c, :], in_=d_xT[c])
    for hp in range(1, FC):
        nc.sync.dma_start(out=wq_sb[:, hp], in_=d_wq[:, hp])
        nc.scalar.dma_start(out=wk_sb[:, hp], in_=d_wk[:, hp])
    for r in range(SQ // P):
        nc.sync.dma_start(out=xres_sb[:, r, :], in_=d_xres[r])

    # ---------------- projection units (fp8 DoubleRow) ----------------
    def unit_q(hp):
        qps = ps_mm.tile([P, SQ], F32, name="qps", tag="mm")
        for c2 in range(FC // 2):
            nc.tensor.matmul(qps, lhsT=wq_sb[:, hp, 2 * c2 : 2 * c2 + 2, :],
                             rhs=xT_sb[:, 2 * c2 : 2 * c2 + 2, 0:SQ],
                             start=(c2 == 0), stop=(c2 == FC // 2 - 1),
                             perf_mode=DR)
        nc.vector.tensor_copy(qT_hp[hp], qps)

    def unit_k(hp, n):
        kps = ps_mm.tile([P, 512], F32, name="kps", tag="mm")
        for c2 in range(FC // 2):
            nc.tensor.matmul(kps, lhsT=wk_sb[:, hp, 2 * c2 : 2 * c2 + 2, :],
                             rhs=xT_sb[:, 2 * c2 : 2 * c2 + 2,
                                       n * 512 : (n + 1) * 512],
                             start=(c2 == 0), stop=(c2 == FC // 2 - 1),
                             perf_mode=DR)
        nc.vector.tensor_copy(kT_hp[hp][:, n * 512 : (n + 1) * 512], kps)

    def unit_v(g, t):
        vps = ps_mm.tile([P, 512], F32, name="vps", tag="mm")
        for c2 in range(FC // 2):
            nc.tensor.matmul(vps, lhsT=xT_sb[:, 2 * c2 : 2 * c2 + 2,
                                            t * P : (t + 1) * P],
                             rhs=wv_sb[:, g, 2 * c2 : 2 * c2 + 2, :],
                             start=(c2 == 0), stop=(c2 == FC // 2 - 1),
                             perf_mode=DR)
        nc.vector.tensor_copy(v_g[g][:, t, :, 0:HD],
                              vps.rearrange("p (h d) -> p h d", d=HD))

    # prefix: everything head pair 0 needs
    unit_q(0)
    for n in range(S // 512):
        unit_k(0, n)
    for t in range(KC):
        unit_v(0, t)

    units = deque()
    vg1 = deque((1, t) for t in range(KC))
    for hp in range(1, FC):
        units.append(("q", hp, 0))
        for n in range(S // 512):
            units.append(("k", hp, n))
        for _ in range(3):
            if vg1:
                units.append(("v", *vg1.popleft()))
    while vg1:
        units.append(("v", *vg1.popleft()))

    def emit_unit():
        if not units:
            return
        kind, a, b = units.popleft()
        if kind == "q":
            unit_q(a)
        elif kind == "k":
            unit_k(a, b)
        else:
            unit_v(a, b)

    # ---------------- attention over head pairs ----------------
    # Software-pipelined: ctx matmuls for step g are emitted at step g+2 so
    # the exp -> mask chain (ACT + DVE/GpSimd) never stalls the PE; a pair's
    # PSUM drain + normalization is emitted after the NEXT pair's scores.
    late = {}
    pend = deque()
    ctx_tiles = {}

    def emit_ctx(e):
        hp_, tg_, pT_ = e
        if tg_ == 0:
            # allocated here (not at pair start) so the previous pair's
            # pending ctx writes + drain keep proper pool generation order
            ctx_tiles[hp_] = (ps_c.tile([P, SQ], F32, name="ctx_psA"),
                              ps_c.tile([P, SQ], F32, name="ctx_psB"))
        cA, cB = ctx_tiles[hp_]
        g_, sa_, sb_ = hp_ // 4, (2 * hp_) % 8, (2 * hp_ + 1) % 8
        for j_ in range(2):
            t_ = tg_ * 2 + j_
            nc.tensor.matmul(
                cA[0 : HD + 1, :], lhsT=v_g[g_][:, t_, sa_, :],
                rhs=pT_[:, j_, 0, :], start=(t_ == 0), stop=(t_ == KC - 1),
            )
            nc.tensor.matmul(
                cB[0 : HD + 1, :], lhsT=v_g[g_][:, t_, sb_, :],
                rhs=pT_[:, j_, 1, :], start=(t_ == 0), stop=(t_ == KC - 1),
            )

    def drain_pair(hp_):
        # drains, then normalize: the ones-row sums bounce through DRAM so a
        # partition-broadcast DMA replicates them across all 128 partitions
        # (keeps normalization entirely off the PE instruction queue)
        cA, cB = ctx_tiles.pop(hp_)
        nc.vector.tensor_copy(ctxT_sb[0:HD, hp_, :], cA[0:HD, :])
        nc.vector.tensor_copy(ctxT_sb[HD:P, hp_, :], cB[0:HD, :])
        stA = work.tile([1, SQ], F32, name="stA")
        stB = work.tile([1, SQ], F32, name="stB")
        nc.vector.tensor_copy(stA, cA[HD : HD + 1, :])
        nc.vector.tensor_copy(stB, cB[HD : HD + 1, :])
        s2 = work.tile([2, SQ], F32, name="s2")
        nc.sync.dma_start(out=s2[0:1, :], in_=stA)
        nc.sync.dma_start(out=s2[1:2, :], in_=stB)
        r2 = work.tile([2, SQ], F32, name="r2")
        nc.vector.reciprocal_approx_fast(r2, s2)
        bc = ps_mm.tile([P, SQ], F32, name="bc", tag="mm")
        nc.tensor.matmul(bc, lhsT=sel2, rhs=r2, start=True, stop=True)
        nc.vector.tensor_mul(ctxF8[:, hp_, :], ctxT_sb[:, hp_, :], bc)

    prev_drain = None
    for hp in range(FC):
        if hp == FC - 1:
            # all projection units emitted; free their SBUF, load dense tiles
            assert not units
            proj_ctx.close()
            late_pool = ctx.enter_context(tc.tile_pool(name="late", bufs=1))
            late["wd"] = late_pool.tile([P, 2, FC, 512], FP8, name="wd_sb")
            nc.scalar.dma_start(out=late["wd"], in_=d_wd)
        for tg in range(KC // 2):
            # slot layout [P, kv-j, head(A/B), SQ] so one mask multiply with a
            # 0-stride broadcast AP covers both heads in a single DVE op
            s_ps = ps_s.tile([P, 2, 2, SQ], F32, name="s_ps")
            for j in range(2):
                t = tg * 2 + j
                nc.tensor.matmul(
                    s_ps[:, j, 0, :],
                    lhsT=kT_hp[hp][0:HD, t * P : (t + 1) * P],
                    rhs=qT_hp[hp][0:HD, :],
                    start=True, stop=True,
                )
                nc.tensor.matmul(
                    s_ps[:, j, 1, :],
                    lhsT=kT_hp[hp][HD:P, t * P : (t + 1) * P],
                    rhs=qT_hp[hp][HD:P, :],
                    start=True, stop=True,
                )
            if tg == 2 and prev_drain is not None:
                drain_pair(prev_drain)
                prev_drain = None
            emit_unit()
            eT = work.tile([P, 2, 2, SQ], BF16, name="eT")
            nc.scalar.activation(eT[:, :, 0, :], s_ps[:, :, 0, :],
                                 mybir.ActivationFunctionType.Exp, scale=SCALE)
            nc.scalar.activation(eT[:, :, 1, :], s_ps[:, :, 1, :],
                                 mybir.ActivationFunctionType.Exp, scale=SCALE)
            pT = work.tile([P, 2, 2, SQ], BF16, name="pT")
            mc = maskT_sb[:, tg * 2 : tg * 2 + 2, :]
            mc_b = bass.AP(tensor=mc.tensor, offset=mc.offset,
                           ap=[list(mc.ap[0]), list(mc.ap[1]), [0, 2],
                               list(mc.ap[2])])
            nc.vector.tensor_mul(pT, eT, mc_b)
            pend.append((hp, tg, pT))
            if len(pend) > 2:
                emit_ctx(pend.popleft())
        prev_drain = hp
    while pend:
        emit_ctx(pend.popleft())
    drain_pair(prev_drain)

    # ---------------- dense + residual + LayerNorm ----------------
    wd_sb = late["wd"]
    ln_pool = ctx.enter_context(tc.tile_pool(name="ln", bufs=2))
    gb_pool = ctx.enter_context(tc.tile_pool(name="gb", bufs=1))
    eps_t = gb_pool.tile([P, 1], F32)
    nc.vector.memset(eps_t, EPS)
    if affine:
        gamma_bc = gb_pool.tile([P, H], F32)
        beta_bc = gb_pool.tile([P, H], F32)
        nc.sync.dma_start(out=gamma_bc, in_=_bcast_ap(d_gamma, P))
        nc.sync.dma_start(out=beta_bc, in_=_bcast_ap(d_beta, P))

    for r in range(SQ // P):
        pre = ln_pool.tile([P, H], F32, name="pre")
        for h2 in range(2):
            dps = ps_mm.tile([P, 512], F32, name="dps", tag="mm")
            for c2 in range(4):
                nc.tensor.matmul(
                    dps,
                    lhsT=ctxF8[:, 2 * c2 : 2 * c2 + 2, r * P : (r + 1) * P],
                    rhs=wd_sb[:, h2, 2 * c2 : 2 * c2 + 2, :],
                    start=(c2 == 0), stop=(c2 == 3),
                    perf_mode=DR,
                )
            nc.vector.tensor_add(pre[:, h2 * 512 : (h2 + 1) * 512], dps,
                                 xres_sb[:, r, h2 * 512 : (h2 + 1) * 512])

        # LayerNorm over free dim (1024) via bn_stats on two 512 subgroups
        stats = ln_pool.tile([P, 2, 6], F32, name="stats")
        nc.vector.bn_stats(stats[:, 0, :], pre[:, 0:512])
        nc.vector.bn_stats(stats[:, 1, :], pre[:, 512:1024])
        mv = ln_pool.tile([P, 2], F32, name="mv")
        nc.vector.bn_aggr(mv, stats)
        std = ln_pool.tile([P, 1], F32, name="std")
        nc.scalar.activation(std, mv[:, 1:2], mybir.ActivationFunctionType.Sqrt,
                             bias=eps_t)
        rstd = ln_pool.tile([P, 1], F32, name="rstd")
        nc.vector.reciprocal(rstd, std)
        outv = ln_pool.tile([P, H], F32, name="outv")
        nc.vector.tensor_scalar(outv, pre, mv[:, 0:1], rstd,
                                mybir.AluOpType.subtract, mybir.AluOpType.mult)
        if affine:
            nc.vector.tensor_mul(outv, outv, gamma_bc)
            nc.vector.tensor_add(outv, outv, beta_bc)
        for piece in range(4):
            eng = nc.sync if piece % 2 == 0 else nc.scalar
            cs = slice(piece * 256, (piece + 1) * 256)
            eng.dma_start(out=d_out[r][:, cs], in_=outv[:, cs])


_nc_cache = {}


def _get_nc(affine):
    if affine not in _nc_cache:
        _nc_cache[affine] = _build_program(affine)
    return _nc_cache[affine]


def kernel(hidden_states, mask, Wq, Wk, Wv, Wd, bd, gamma, beta):
    global last_results
    hidden_states = np.asarray(hidden_states, dtype=np.float32)
    mask = np.asarray(mask)
    Wq = np.asarray(Wq, dtype=np.float32)
    Wk = np.asarray(Wk, dtype=np.float32)
    Wv = np.asarray(Wv, dtype=np.float32)
    Wd = np.asarray(Wd, dtype=np.float32)
    bd = np.asarray(bd, dtype=np.float32)
    gamma = np.asarray(gamma, dtype=np.float32)
    beta = np.asarray(beta, dtype=np.float32)

    affine = bool(np.any(gamma != 1.0) or np.any(beta != 0.0))
    nc = _get_nc(affine)

    sel_np = np.zeros((2, P), dtype=np.float32)
    sel_np[0, 0:HD] = 1.0
    sel_np[1, HD:P] = 1.0

    # weights pre-arranged into their exact SBUF images (fp8)
    wqT = np.ascontiguousarray(
        Wq.reshape(FC, P, FC, P).transpose(3, 0, 2, 1)).astype(NP_FP8)
    wkT = np.ascontiguousarray(
        Wk.reshape(FC, P, FC, P).transpose(3, 0, 2, 1)).astype(NP_FP8)
    wvT = np.ascontiguousarray(
        Wv.reshape(2, 512, FC, P).transpose(3, 0, 2, 1)).astype(NP_FP8)
    wdT = np.ascontiguousarray(
        Wd.reshape(2, 512, FC, P).transpose(3, 0, 2, 1)).astype(NP_FP8)

    in_maps = []
    for c in range(NCORES):
        b, qi = c // 4, c % 4
        qs = qi * SQ
        # roll the kv axis so this core's own query rows are columns 0..SQ
        xT = np.roll(hidden_states[b].T, -qs, axis=1)
        xT = np.ascontiguousarray(xT).astype(NP_FP8).reshape(FC, P, S)
        maskT = np.roll(mask[b].T, -qs, axis=0)[:, qs : qs + SQ]
        maskT = np.ascontiguousarray(maskT).astype(NP_BF16).reshape(KC, P, SQ)
        xres = (hidden_states[b, qs : qs + SQ] + bd[None, :]).astype(np.float32)
        in_maps.append({
            "xT": xT,
            "wqT": wqT,
            "wkT": wkT,
            "wvT": wvT,
            "wdT": wdT,
            "maskT": maskT,
            "xres": np.ascontiguousarray(xres.reshape(SQ // P, P, H)),
            "gamma": gamma,
            "beta": beta,
            "sel": sel_np,
        })

    trace = os.environ.get("BASS_KERNEL_TRACE", "0") == "1"
    res = run_bass_kernel_spmd(
        nc, in_maps, core_ids=list(range(NCORES)), trace=trace
    )
    last_results = res

    out = np.empty((B, S, H), dtype=np.float32)
    for c in range(NCORES):
        b, qi = c // 4, c % 4
        out[b, qi * SQ : (qi + 1) * SQ] = res.results[c]["out"].reshape(SQ, H)
    return out
